# revision 42
# baseline (speedup 1.0000x reference)
"""MHSA (RoPE + causal softmax) Trainium2 Bass kernel.

Problem: x[4,2048,1024], Wq/Wk/Wv/Wo[1024,1024] fp32; 16 heads, d_k=64.

Sharding over the 8 NeuronCores: 4-way data-parallel over batch x 2-way
tensor-parallel over heads. core = 2*b + t handles batch b, heads
[t*8, t*8+8). Column-parallel Wq/Wk/Wv, row-parallel Wo; the two TP
partial outputs per batch are summed on the host (the gather step).

Device-side design (per core, all sizes hardcoded):
  - Host supplies x^T [1024,2048] (bf16) so every matmul contraction dim
    (model dim d or k-positions) lands on SBUF partitions. All matmuls
    are bf16 (1 cycle/row, fp32 PSUM accumulation); measured end-to-end
    max-relative error vs the fp32 reference is ~4e-3.
  - RoPE: interleaved even/odd pairs are pre-permuted in the Wq/Wk ROWS
    (host side) so each rotation partner lives 16 partitions away within
    a 32-partition quadrant; the rotation is then
        q' = q * cosT + stream_shuffle(q) * sinT
    with a single DVE stream-shuffle (swap 16-blocks) and sign baked
    into sinT. Scores are permutation-invariant so nothing downstream
    changes.
  - Attention per head pair (2 heads stacked on 128 partitions,
    auto row-tiled matmuls at base partitions 0/64):
       S^T[j] = K_j @ Q_I^T          (bf16, [128 kpos, <=512 q])
       P^T    = exp(S^T / 8)          (ACT, psum -> bf16 sbuf)
       causal: block-skip j>4I+3, narrow diagonal tiles, one [128,128]
       tri-mask multiply on the diagonal block
       O^T   += [V_j | 1x64]^T @ P^T  (bf16; V carries 64 ones-columns so
                                       rows 64..127 of O^T replicate the
                                       softmax denominator - no
                                       max-subtraction softmax, safe for
                                       these magnitudes, and no partition
                                       broadcast needed)
       mh^T   = O^T[0:64] * approx_recip(O^T[64:128])
  - Output projection back in [s, d] layout (lhsT = mh^T tiles) so the
    DRAM write is contiguous; host sums the two TP partials per batch.
  - Schedule: first-needed DMA chunks (x block 0 + Wq column block 0)
    lead the issue queues, ordered by first use (x/Wq -> cos/sin -> Wk
    -> Wv) so the PE starts within ~2us of the framework barrier; a
    short PE warm-up bridges the DMA-bound preamble (and holds the HAM
    clock gate at full rate - shrinking it measurably slows every
    matmul). Projection/outproj matmul groups for the next block are
    interleaved into each attention pair between its scores tiles and
    before its AV phase, and each pair pre-emits the first 4 scores
    tiles of the NEXT pair ("exp lookahead") so the ACT exp chain is
    always running while AV matmuls drain - the exp chain, not the PE,
    is the per-pair critical path otherwise. The final pair splits its
    AV into two column groups so the softmax divide of the first half
    overlaps the second half's matmuls, and the final output projection
    round-robins its PSUM tiles across two pools (4-deep ring) with
    PSUM->SBUF copies split across ACT and DVE. Output partials are
    written bf16 (host upcasts and sums the TP pair in fp32).
"""
import numpy as np
import ml_dtypes

import concourse.bass as bass
from concourse import bacc
import concourse.tile as tile
import concourse.mybir as mybir
from concourse.bass_utils import run_bass_kernel_spmd

B, S, D = 4, 2048, 1024
HEADS, DK = 16, 64
THETA = 10000.0
TP, DP = 2, 4
HL = HEADS // TP            # 8 local heads per core
DL = HL * DK                # 512 local projection width
P = 128
SB = 512                    # q super-tile width
NSB = S // SB               # 4 q super-tiles (I)
NST = S // P                # 16 k-tiles (j)
NDC = D // P                # 8 contraction chunks over model dim
NPAIR = HL // 2             # 4 local head pairs

f32 = mybir.dt.float32
f32r = mybir.dt.float32r
bf16 = mybir.dt.bfloat16
SWAP16 = [(i + 16) % 32 for i in range(32)]


def _build():
    ALU = mybir.AluOpType
    FX = mybir.ActivationFunctionType
    nc = bacc.Bacc(None, target_bir_lowering=False)

    xT = nc.dram_tensor("xT", [D, S], bf16, kind="ExternalInput")
    wqkvT = nc.dram_tensor("wqkvT", [D, 3 * DL], bf16, kind="ExternalInput")
    woT = nc.dram_tensor("woT", [DL, D], bf16, kind="ExternalInput")
    cosf = nc.dram_tensor("cosf", [P, S], bf16, kind="ExternalInput")
    sinf = nc.dram_tensor("sinf", [P, S], bf16, kind="ExternalInput")
    maskt = nc.dram_tensor("maskt", [P, P], bf16, kind="ExternalInput")
    out = nc.dram_tensor("out", [S, D], bf16, kind="ExternalOutput")

    xT_t = xT.rearrange("(dc p) s -> p dc s", p=P)        # [128, 8, 2048]
    wq_t = wqkvT.rearrange("(dc p) f -> p dc f", p=P)     # [128, 8, 1536]
    wo_t = woT.rearrange("(c p) f -> p c f", p=P)         # [128, 4, 1024]

    with tile.TileContext(nc) as tc:
        with (
            tc.tile_pool(name="wpool", bufs=1) as wpool,
            tc.tile_pool(name="kpool", bufs=1) as kpool,
            tc.tile_pool(name="xpool", bufs=2) as xpool,
            tc.tile_pool(name="qpool", bufs=2) as qpool,
            tc.tile_pool(name="tpool", bufs=2) as tpool,
            tc.tile_pool(name="ptpool", bufs=4) as ptpool,
            tc.tile_pool(name="mpool", bufs=3) as mpool,
            tc.tile_pool(name="spool", bufs=2) as spool,
            tc.tile_pool(name="opool", bufs=2) as opool,
            tc.tile_pool(name="ps_proj", bufs=2, space="PSUM") as ps_proj,
            tc.tile_pool(name="ps_s", bufs=2, space="PSUM") as ps_s,
            tc.tile_pool(name="ps_av", bufs=1, space="PSUM") as ps_av,
        ):
            # PE warm-up source first: the DVE is idle at t=0 and the
            # warm matmuls then start within ~1us, holding the HAM clock
            # gate at 2.4GHz through the DMA-bound preamble.
            warm = wpool.tile([P, SB], bf16)
            nc.vector.memset(warm[:], 0.0)

            # First-needed DMA chunks lead all three issue queues: the
            # block-0 x chunks and the Wq column slices (gating the first
            # matmul group), strictly alternating so dc=0 lands first.
            # Rings dedicated by need-time: x chunks stream on gpsimd
            # (first matmul group), Q then K weights on sync, cos/sin +
            # V weights on scalar. Keeps each ring's cumulative bytes
            # below its consumer's start time.
            xt0 = xpool.tile([P, NDC, SB], bf16, tag="xt", name="xt0")
            w_sb = wpool.tile([P, NDC, 3 * DL], bf16)
            cos_sb = wpool.tile([P, S], bf16)
            sin_sb = wpool.tile([P, S], bf16)
            for dc in range(NDC):
                nc.gpsimd.dma_start(xt0[:, dc, :], xT_t[:, dc, 0:SB])
                eng = nc.sync if dc < 4 else nc.scalar
                eng.dma_start(w_sb[:, dc, 0:DL], wq_t[:, dc, 0:DL])
            nc.scalar.dma_start(cos_sb[:, 0:SB], cosf[:, 0:SB])
            nc.scalar.dma_start(sin_sb[:, 0:SB], sinf[:, 0:SB])
            nc.sync.dma_start(w_sb[:, 0:4, DL : 2 * DL], wq_t[:, 0:4, DL : 2 * DL])
            nc.sync.dma_start(w_sb[:, 4:8, DL : 2 * DL], wq_t[:, 4:8, DL : 2 * DL])
            nc.scalar.dma_start(
                w_sb[:, :, 2 * DL : 3 * DL], wq_t[:, :, 2 * DL : 3 * DL]
            )
            nc.sync.dma_start(cos_sb[:, SB:S], cosf[:, SB:S])
            nc.scalar.dma_start(sin_sb[:, SB:S], sinf[:, SB:S])
            mask_sb = wpool.tile([P, P], bf16)
            nc.gpsimd.dma_start(mask_sb[:], maskt[:])
            wo_sb = wpool.tile([P, DL // P, D], bf16)

            ktall = kpool.tile([P, NPAIR, S], bf16)
            v_sb = kpool.tile([P, NST, HL, 2 * DK], bf16)
            nc.gpsimd.memset(v_sb[:, :, :, DK : 2 * DK], 1.0)

            def emit_warm(n):
                pw = ps_s.tile([P, 2, SB], f32, tag="pss", name=f"warm{emit_warm.k}")
                emit_warm.k += 1
                for r in range(n):
                    nc.tensor.matmul(pw[:, 0, :], warm[:, 0:P], warm[:],
                                     start=True, stop=True)
                sink = tpool.tile([P, SB], f32, tag="tsh", name="warmsink")
                nc.vector.tensor_copy(sink[:], pw[:, 0, :])
            emit_warm.k = 0
            emit_warm(14)

            def emit_outproj(Io, mh, sts=None, act_copy=False):
                # row-parallel partial output projection for q block Io.
                # act_copy (final block): pq tiles alternate between the
                # ps_proj and the now-idle ps_s pools (4-deep ring) and
                # each PSUM->SBUF copy splits across ACT and DVE, so the
                # tail is matmul-limited rather than copy-limited.
                for st in (range(SB // P) if sts is None else sts):
                    osb = opool.tile([P, D], bf16, tag="osb", name=f"osb{Io}_{st}")
                    for oh in (0, 1):
                        if act_copy and (2 * st + oh) % 2 == 1:
                            pq = ps_s.tile(
                                [P, SB], f32, tag="pss", name=f"pq{Io}_{st}_{oh}"
                            )
                        else:
                            pq = ps_proj.tile(
                                [P, SB], f32, tag="pp", name=f"pq{Io}_{st}_{oh}"
                            )
                        for c2 in range(NPAIR):
                            nc.tensor.matmul(
                                pq[:],
                                mh[:, c2, st * P : (st + 1) * P],
                                wo_sb[:, c2, oh * SB : (oh + 1) * SB],
                                start=(c2 == 0),
                                stop=(c2 == NPAIR - 1),
                            )
                        od = osb[:, oh * SB : (oh + 1) * SB]
                        if act_copy:
                            hw = SB // 2
                            nc.scalar.copy(od[:, 0:hw], pq[:, 0:hw])
                            nc.vector.tensor_copy(od[:, hw:SB], pq[:, hw:SB])
                        else:
                            nc.vector.tensor_copy(od, pq[:])
                        nc.sync.dma_start(
                            out[
                                (Io * 4 + st) * P : (Io * 4 + st + 1) * P,
                                oh * SB : (oh + 1) * SB,
                            ],
                            od,
                        )

            def emit_qk_group(I, fc, xt, qt):
                scol = slice(I * SB, (I + 1) * SB)
                pp = ps_proj.tile([P, SB], f32, tag="pp", name=f"pp{I}_{fc}")
                for dc in range(NDC):
                    nc.tensor.matmul(
                        pp[:],
                        w_sb[:, dc, fc * P : (fc + 1) * P],
                        xt[:, dc, :],
                        start=(dc == 0),
                        stop=(dc == NDC - 1),
                    )
                tsh = tpool.tile([P, SB], f32, tag="tsh", name=f"tsh{I}_{fc}")
                nc.vector.stream_shuffle(tsh[:], pp[:], mask=SWAP16)
                dest = qt[:, fc, :] if fc < NPAIR else ktall[:, fc - NPAIR, scol]
                nc.vector.tensor_tensor(dest, pp[:], cos_sb[:, scol], ALU.mult)
                tsn = tpool.tile([P, SB], f32, tag="tsn", name=f"tsn{I}_{fc}")
                nc.vector.tensor_tensor(tsn[:], tsh[:], sin_sb[:, scol], ALU.mult)
                nc.vector.tensor_tensor(dest, dest, tsn[:], ALU.add)

            def emit_v_group(I, st, xt):
                pp = ps_proj.tile([P, SB], f32, tag="pp", name=f"ppv{I}_{st}")
                for dc in range(NDC):
                    nc.tensor.matmul(
                        pp[:],
                        xt[:, dc, st * P : (st + 1) * P],
                        w_sb[:, dc, 2 * DL : 3 * DL],
                        start=(dc == 0),
                        stop=(dc == NDC - 1),
                    )
                nc.scalar.copy(
                    v_sb[:, I * 4 + st, :, 0:DK],
                    pp[:].rearrange("p (h d) -> p h d", h=HL),
                )

            def emit_mh_fin(I, c, mhI, po0, po1, a, b, act_lsb=False):
                # softmax divide for q columns [a:b) of pair c
                for half, po in ((0, po0), (1, po1)):
                    pr = 64 * half
                    lsb = spool.tile(
                        [64, SB], f32, tag="lsb", name=f"lsb{I}_{c}_{half}_{a}"
                    )
                    (nc.scalar.copy if act_lsb else nc.vector.tensor_copy)(
                        lsb[:, a:b], po[DK:P, a:b]
                    )
                    rec = spool.tile(
                        [64, SB], f32, tag="rec", name=f"rec{I}_{c}_{half}_{a}"
                    )
                    nc.vector.reciprocal_approx_fast(rec[:, a:b], lsb[:, a:b])
                    nc.vector.tensor_tensor(
                        mhI[pr : pr + 64, c, a:b], po[0:DK, a:b], rec[:, a:b],
                        ALU.mult,
                    )

            def emit_scores_tiles(I, c, qt, js, pts):
                for j in js:
                    m = j - 4 * I
                    off = m * P if m > 0 else 0
                    N = SB - off
                    pss = ps_s.tile([P, 2, SB], f32, tag="pss", name=f"pss{I}_{c}_{j}")
                    for half in (0, 1):
                        pr = 64 * half
                        nc.tensor.matmul(
                            pss[:, half, :N],
                            ktall[pr : pr + 64, c, j * P : (j + 1) * P],
                            qt[pr : pr + 64, c, off:SB],
                            start=True,
                            stop=True,
                        )
                    pt = ptpool.tile(
                        [P, 2, SB], bf16, tag="pt", name=f"pt{I}_{c}_{j}",
                        bufs=NST + 4,
                    )
                    nc.scalar.activation(
                        pt[:, :, :N], pss[:, :, :N], FX.Exp, scale=0.125
                    )
                    if m >= 0:
                        nc.vector.tensor_tensor(
                            pt[:, :, 0:P], pt[:, :, 0:P],
                            mask_sb[:, None, :].to_broadcast((P, 2, P)),
                            ALU.mult,
                        )
                    pts[j] = (pt, off, N)

            def emit_attn_pair(I, c, qt, mhI, mid=(), split_av=False,
                               pts_pre=None, look=None):
                njt = 4 * I + 4
                pts = pts_pre if pts_pre is not None else {}
                po0 = ps_av.tile([P, SB], f32, tag="po0", name=f"po{I}_{c}_0", bufs=1)
                po1 = ps_av.tile([P, SB], f32, tag="po1", name=f"po{I}_{c}_1", bufs=1)
                mid = list(mid)
                todo = [j for j in range(njt) if j not in pts]
                for idx, j in enumerate(todo):
                    emit_scores_tiles(I, c, qt, [j], pts)
                    if idx % 2 == 1 and mid:
                        mid.pop(0)()
                for fn in mid:
                    fn()
                # next pair's first scores tiles: starts its exp chain on
                # the ACT engine while this pair's AV matmuls run
                if look is not None:
                    look()
                # Column groups: split_av emits AV in two independent
                # accumulation groups over q columns so the first half's
                # softmax divide (DVE) overlaps the second half's
                # matmuls - shrinks the tail before the last outproj.
                cgs = [(0, SB // 2), (SB // 2, SB)] if split_av else [(0, SB)]
                for a, bnd in cgs:
                    js = [j for j in range(njt) if (max(j - 4 * I, 0) * P) < bnd]
                    for ji, j in enumerate(js):
                        pt, off, N = pts[j]
                        lo = max(off, a)
                        for half, po in ((0, po0), (1, po1)):
                            nc.tensor.matmul(
                                po[:, lo:bnd],
                                v_sb[:, j, 2 * c + half, :],
                                pt[:, half, lo - off : bnd - off],
                                start=(ji == 0),
                                stop=(ji == len(js) - 1),
                                skip_group_check=True,
                            )
                    emit_mh_fin(I, c, mhI, po0, po1, a, bnd, act_lsb=split_av and a > 0)

            # Software pipeline: iteration i emits projection work for block
            # i interleaved with attention pairs of block i-1. One proj
            # group leads each pair; two more plus the previous block's
            # outproj tile run between the pair's scores and AV phases.
            qts, mhs = {}, {}
            oq = []
            for i in range(NSB + 1):
                work = []
                if i < NSB:
                    if i == 0:
                        xt = xt0
                    else:
                        xt = xpool.tile([P, NDC, SB], bf16, tag="xt", name=f"xt{i}")
                        for dc in range(NDC):
                            eng = nc.sync if dc % 2 == 0 else nc.gpsimd
                            eng.dma_start(
                                xt[:, dc, :], xT_t[:, dc, i * SB : (i + 1) * SB]
                            )
                    qt = qpool.tile([P, NPAIR, SB], bf16, tag="qt", name=f"qt{i}")
                    qts[i] = qt
                    work += [
                        (lambda fc=fc, I=i, xt=xt, qt=qt: emit_qk_group(I, fc, xt, qt))
                        for fc in range(2 * NPAIR)
                    ]
                    work += [
                        (lambda st=st, I=i, xt=xt: emit_v_group(I, st, xt))
                        for st in range(SB // P)
                    ]
                Ia = i - 1
                wi = 0
                if Ia >= 0:
                    mhI = mpool.tile([P, NPAIR, SB], bf16, tag="mh", name=f"mh{Ia}")
                    mhs[Ia] = mhI
                    # outproj tiles pop from a deferral queue: lighter in
                    # iters 2-3 (ACT has slack there), heavier in the
                    # proj-free last iter so its pairs outrun the exp chain
                    pops = {2: (1, 1, 1, 1), 3: (1, 1, 0, 0), 4: (1, 1, 1, 3)}
                    nxt_pts = {}
                    for c in range(NPAIR):
                        mid = []
                        for _ in range(3):
                            if wi < len(work):
                                mid.append(work[wi])
                                wi += 1
                        for _ in range(pops.get(i, (0,) * 4)[c]):
                            if oq:
                                mid.append(oq.pop(0))
                        cur_pts, nxt_pts = nxt_pts, {}
                        look = None
                        if c + 1 < NPAIR:
                            look = (
                                lambda c1=c + 1, d=nxt_pts: emit_scores_tiles(
                                    Ia, c1, qts[Ia], range(min(4, 4 * Ia + 4)), d
                                )
                            )
                        last = Ia == NSB - 1 and c == NPAIR - 1
                        emit_attn_pair(
                            Ia, c, qts[Ia], mhI, mid=mid, split_av=last,
                            pts_pre=cur_pts, look=look,
                        )
                        if last:
                            while oq:
                                oq.pop(0)()
                    if Ia < NSB - 1:
                        for st in range(SB // P):
                            oq.append(
                                lambda st=st, Ia=Ia: emit_outproj(
                                    Ia, mhs[Ia], sts=(st,)
                                )
                            )
                while wi < len(work):
                    work[wi]()
                    wi += 1
                if i == 0:
                    nc.gpsimd.dma_start(wo_sb[:], wo_t[:])
            emit_outproj(NSB - 1, mhs[NSB - 1], act_copy=True)
    nc.finalize()
    return nc


_NC = None


def _get_nc():
    global _NC
    if _NC is None:
        _NC = _build()
    return _NC


def _host_prep(Wq, Wk, Wv, Wo):
    t = np.arange(DK // 2)
    qd, rd = t // 16, t % 16
    perm = np.empty(DK, np.int64)
    perm[qd * 32 + rd] = 2 * t
    perm[qd * 32 + 16 + rd] = 2 * t + 1

    Wq_p = Wq.reshape(HEADS, DK, D)[:, perm, :].reshape(HEADS * DK, D)
    Wk_p = Wk.reshape(HEADS, DK, D)[:, perm, :].reshape(HEADS * DK, D)

    pos = np.arange(S, dtype=np.float64)
    inv = 1.0 / THETA ** (np.arange(0, DK, 2).astype(np.float64) / DK)  # [32]
    ang = inv[:, None] * pos[None, :]                                   # [32, S]
    cos32 = np.cos(ang).astype(np.float32)
    sin32 = np.sin(ang).astype(np.float32)
    cosf = np.empty((DK, S), np.float32)
    sinf = np.empty((DK, S), np.float32)
    rows_lo = qd * 32 + rd
    rows_hi = qd * 32 + 16 + rd
    cosf[rows_lo] = cos32[t]
    cosf[rows_hi] = cos32[t]
    sinf[rows_lo] = -sin32[t]
    sinf[rows_hi] = sin32[t]
    # duplicate across both 64-partition halves -> single [128, S] DMA
    cosf = np.concatenate([cosf, cosf], axis=0)
    sinf = np.concatenate([sinf, sinf], axis=0)

    mask01 = (
        np.arange(P)[:, None] <= np.arange(P)[None, :]
    ).astype(ml_dtypes.bfloat16)

    per_tp = []
    for tp in range(TP):
        sl = slice(tp * DL, (tp + 1) * DL)
        wqkvT = np.ascontiguousarray(
            np.concatenate([Wq_p[sl], Wk_p[sl], Wv[sl]], axis=0).T
        ).astype(ml_dtypes.bfloat16)
        woT = np.ascontiguousarray(Wo[:, sl].T).astype(ml_dtypes.bfloat16)
        per_tp.append((wqkvT, woT))
    return per_tp, cosf.astype(ml_dtypes.bfloat16), sinf.astype(ml_dtypes.bfloat16), mask01


def kernel(x, Wq, Wk, Wv, Wo):
    x = np.asarray(x, np.float32)
    Wq = np.asarray(Wq, np.float32)
    Wk = np.asarray(Wk, np.float32)
    Wv = np.asarray(Wv, np.float32)
    Wo = np.asarray(Wo, np.float32)

    per_tp, cosf, sinf, mask01 = _host_prep(Wq, Wk, Wv, Wo)
    xTs = [np.ascontiguousarray(x[b].T).astype(ml_dtypes.bfloat16) for b in range(B)]

    in_maps = []
    for core in range(DP * TP):
        b, tp = core // TP, core % TP
        wqkvT, woT = per_tp[tp]
        in_maps.append(
            {
                "xT": xTs[b],
                "wqkvT": wqkvT,
                "woT": woT,
                "cosf": cosf,
                "sinf": sinf,
                "maskt": mask01,
            }
        )

    nc = _get_nc()
    res = run_bass_kernel_spmd(nc, in_maps, core_ids=list(range(DP * TP)))
    out = np.empty((B, S, D), np.float32)
    for b in range(B):
        out[b] = res.results[b * TP]["out"].astype(np.float32) + res.results[
            b * TP + 1
        ]["out"].astype(np.float32)
    return out
